# revision 36
# baseline (speedup 1.0000x reference)
"""Trainium2 Bass kernel for nn_LiquidGenerator.

score = sum over (i, image j) pairs of (CUTOFF - dist)^2 where dist < CUTOFF,
with dist over the [N, 27N] supercell distance matrix.

Strategy
--------
Host (O(N) prep): generate P (rotation+translation of molecule-local coords,
float64), exploit the shift symmetry d(i,(k,j)) == d(j,(-k,i)):
    score = sum_full(central) + 2 * sum_full(13 half-shifts)
so only 14 of the 27 images are computed. Distances come from a 5-feature
inner product:  d^2 + BIAS = [Px,Py,Pz,|P|^2,1] . [-2Sx,-2Sy,-2Sz, 1, |S|^2+BIAS]
(coordinates centered at the cell midpoint for fp32 accuracy; BIAS keeps the
PE-accumulated value strictly positive so ACT Sqrt never sees a negative).

Device (8 NeuronCores, j-sharded): each core takes 128 columns of each of the
14 shift blocks (1792 cols) for all 1024 i. Per 128-row i-chunk:
  - TensorE: 4 fp32 matmuls (K=5) -> PSUM [128,1792] holding d^2+BIAS
  - ScalarE: Sqrt activation PSUM->SBUF
  - VectorE: one fused custom-DVE op per weight group:
        sq(relu(CUTOFF - s)) * ne(Idx, diag) * weight, accum=ADD
    (free reduction to [128,1]; diag masks the self-pairs, which are added
    back exactly on the host).
Host: fp64 sum of the 8x[128,16] partials + exact self-pair term.
"""

import numpy as np

CUTOFF = 3.0
EPS = 1e-16
BIAS = 2e-4

NCORES = 8
N = 1024           # 128 molecules x 8 atoms
NSH = 14           # central + 13 half-shifts
JC = N // NCORES   # 128 j-columns per shift block per core
W = NSH * JC       # 1792 columns per core
NCHUNK = 8         # i-chunks of 128

_OP_NAME = "LIQUID_SCORE_ANT"
_cache: dict = {}


# ----------------------------------------------------------------- host math
def _rotation_matrices(rot):
    a, b, g = rot[:, 0], rot[:, 1], rot[:, 2]
    ca, sa = np.cos(a), np.sin(a)
    cb, sb = np.cos(b), np.sin(b)
    cg, sg = np.cos(g), np.sin(g)
    m = rot.shape[0]
    rx = np.zeros((m, 3, 3)); ry = np.zeros((m, 3, 3)); rz = np.zeros((m, 3, 3))
    rx[:, 0, 0] = 1;  rx[:, 1, 1] = ca; rx[:, 1, 2] = -sa; rx[:, 2, 1] = sa; rx[:, 2, 2] = ca
    ry[:, 0, 0] = cb; ry[:, 0, 2] = -sb; ry[:, 1, 1] = 1;  ry[:, 2, 0] = sb; ry[:, 2, 2] = cb
    rz[:, 0, 0] = cg; rz[:, 0, 1] = -sg; rz[:, 1, 0] = sg; rz[:, 1, 1] = cg; rz[:, 2, 2] = 1
    return np.einsum("mij,mjk,mkl->mil", rx, ry, rz)


def _generate(positions, translation, rotation, cell):
    R = _rotation_matrices(rotation.astype(np.float64))
    trans = np.remainder(translation.astype(np.float64), 1.0) @ cell.astype(np.float64)
    gen = np.einsum("mai,mij->maj", positions.astype(np.float64), R) + trans[:, None, :]
    return gen.reshape(-1, 3)


# ------------------------------------------------------------- bass program
def _build_program(reps: int = 1):
    key = ("nc", reps)
    if key in _cache:
        return _cache[key]
    from contextlib import ExitStack
    import concourse.bass as bass  # noqa: F401
    import concourse.tile as tile
    from concourse import bacc, mybir

    f32 = mybir.dt.float32
    # Bacc (not raw Bass): its compile() runs the wait-legalization passes
    # (move_matmul_waits_to_ldweights / generate_event_semaphores) that split
    # multi-semaphore waits, plus ACT table-load insertion.
    nc = bacc.Bacc("TRN2", target_bir_lowering=False, debug=False, num_devices=NCORES)
    # lhsT [5, N] and rhs [5, W] bundled: a single DMA -> a single sync wait on
    # the first matmul (a matmul can encode only ONE sync wait; two DMA sems
    # overflow it at walrus codegen). Tail of 64 zero columns doubles as the
    # bf16-zero operand pool for the toucher matmuls.
    feat_d = nc.dram_tensor("feat", [5, N + W + 64], f32, kind="ExternalInput")
    acc_d = nc.dram_tensor("acc", [128, 2 * NCHUNK], f32, kind="ExternalOutput")
    sdiag_d = nc.dram_tensor("sdiag", [128, NCHUNK * JC], f32, kind="ExternalOutput")

    with tile.TileContext(nc) as tc, ExitStack() as ctx:
        const = ctx.enter_context(tc.tile_pool(name="const", bufs=1))
        psum = ctx.enter_context(tc.tile_pool(name="psum", bufs=2, space="PSUM"))
        # one st buffer per chunk: no SBUF reuse -> no cross-engine WAR deps
        # (this walrus build allows at most ONE sync wait per instruction)
        spool = ctx.enter_context(tc.tile_pool(name="s", bufs=NCHUNK))
        scrap = ctx.enter_context(tc.tile_pool(name="scrap", bufs=2))

        ft = const.tile([5, N + W + 64], f32)
        nc.sync.dma_start(ft[:], feat_d[:])
        lt = ft[:, 0:N]
        rt = ft[:, N:N + W]
        at = const.tile([128, 2 * NCHUNK], f32)
        sall = const.tile([128, NCHUNK * JC], f32)

        # bf16-zero views of the zero-padded feat tail for "toucher" matmuls
        bw = ft[0:1, N + W:N + W + 64].bitcast(mybir.dt.bfloat16)  # [1,128] bf16 zeros
        bx = bw[:, 0:1]

        for ic in range(NCHUNK * reps):
            ic = ic % NCHUNK
            ps = psum.tile([128, W], f32)
            for o in range(0, W, 512):
                w = min(512, W - o)
                nc.tensor.matmul(
                    ps[:, o:o + w],
                    lt[:, ic * 128:(ic + 1) * 128],
                    rt[:, o:o + w],
                    start=True, stop=True,
                )
            st = spool.tile([128, W], f32)
            nc.scalar.activation(st[:], ps[:], mybir.ActivationFunctionType.Sqrt)
            # Toucher: after ACT has read the PSUM tile, a 1-column bf16 matmul
            # re-takes ownership on the PE with a single ACT wait. Walrus allows
            # at most ONE sync wait per matmul; without this, the next chunk's
            # fp32 matmul would need two (ACT done + PE completion).
            nc.tensor.matmul(ps[:, 0:1], bw, bx, start=True, stop=True)
            # stash central s block for exact host-side self-pair correction
            # (GPSIMD is otherwise idle; >8 DMAs would reuse HW queues and need
            # a second sync wait, which walrus rejects)
            nc.gpsimd.tensor_copy(sall[:, ic * JC:(ic + 1) * JC], st[:, 0:JC])
            # v = min(s, 3) - 3  ->  v^2 == relu(3-s)^2
            vt = scrap.tile([128, W], f32)
            nc.vector.tensor_scalar(
                vt[:], st[:], CUTOFF, CUTOFF,
                mybir.AluOpType.min, mybir.AluOpType.subtract,
            )
            sq = scrap.tile([128, W], f32, tag="sqout")
            # central block (weight 1 on host)
            nc.vector.scalar_tensor_tensor(
                sq[:, 0:JC], vt[:, 0:JC], 1.0, vt[:, 0:JC],
                mybir.AluOpType.mult, mybir.AluOpType.mult,
                accum_out=at[:, 2 * ic:2 * ic + 1],
            )
            # 13 half-shift blocks (weight 2 on host, via shift symmetry)
            nc.vector.scalar_tensor_tensor(
                sq[:, JC:W], vt[:, JC:W], 1.0, vt[:, JC:W],
                mybir.AluOpType.mult, mybir.AluOpType.mult,
                accum_out=at[:, 2 * ic + 1:2 * ic + 2],
            )
        nc.sync.dma_start(acc_d[:], at[:])
        nc.sync.dma_start(sdiag_d[:], sall[:])

    # Bacc.finalize runs compile(): wait legalization (one sync wait per
    # instruction on this walrus), ACT table loads, register allocation.
    nc.finalize()
    _cache[key] = nc
    return nc


# --------------------------------------------------------------- input prep
def _prepare_inputs(positions, translation, rotation, cell):
    cell64 = cell.astype(np.float64)
    P = _generate(positions, translation, rotation, cell64)      # [N,3] float64
    n = P.shape[0]
    assert n == N, f"kernel hardcodes N={N}, got {n}"

    shifts = np.array([-1.0, 0.0, 1.0])
    offs = np.stack(np.meshgrid(shifts, shifts, shifts, indexing="ij")).reshape(3, -1).T
    vecs = offs @ cell64                                          # [27,3]
    assert np.all(offs[13] == 0.0)
    used = [13] + [k for k in range(13)]                          # central first

    c = 0.5 * cell64.sum(axis=0)
    Pc = (P - c).astype(np.float32)
    lhsT = np.stack([
        Pc[:, 0], Pc[:, 1], Pc[:, 2],
        (Pc.astype(np.float64) ** 2).sum(1).astype(np.float32),
        np.ones(n, np.float32),
    ]).astype(np.float32)                                         # [5, N]

    rhs_blocks = []
    for k in used:
        S = (P + vecs[k] - c).astype(np.float32)
        rhs_blocks.append(np.stack([
            -2.0 * S[:, 0], -2.0 * S[:, 1], -2.0 * S[:, 2],
            np.ones(n, np.float32),
            (S.astype(np.float64) ** 2).sum(1).astype(np.float32) + np.float32(BIAS),
        ]).astype(np.float32))                                    # [5, N] each

    in_maps = []
    for core in range(NCORES):
        sl = slice(core * JC, (core + 1) * JC)
        rhs = np.concatenate([b[:, sl] for b in rhs_blocks], axis=1)  # [5, W]
        in_maps.append({
            "feat": np.ascontiguousarray(np.concatenate(
                [lhsT, rhs, np.zeros((5, 64), np.float32)], axis=1)),
        })
    return in_maps


LAST_RESULTS = None


def kernel(positions, translation, rotation, cell, _reps=1):
    global LAST_RESULTS
    from concourse.bass_utils import run_bass_kernel_spmd

    nc = _build_program(reps=_reps)
    in_maps = _prepare_inputs(
        np.asarray(positions), np.asarray(translation),
        np.asarray(rotation), np.asarray(cell),
    )
    res = run_bass_kernel_spmd(nc, in_maps, core_ids=list(range(NCORES)))
    LAST_RESULTS = res
    total = 0.0
    for core, r in enumerate(res.results):
        acc = r["acc"].astype(np.float64)
        total += acc[:, 0::2].sum() + 2.0 * acc[:, 1::2].sum()
        # exact removal of the device-computed self-pair terms: the diagonal of
        # this core's central block, recomputed from the exact s values
        s_pp = r["sdiag"][:, core * JC:(core + 1) * JC].diagonal()
        v = (np.minimum(s_pp, np.float32(CUTOFF)) - np.float32(CUTOFF)).astype(np.float32)
        total -= (v.astype(np.float64) ** 2).sum()
    total += N * (CUTOFF - np.sqrt(np.float32(EPS))) ** 2  # exact self pairs
    return np.float32(total)


# revision 38
# speedup vs baseline: 8138.6861x; 8138.6861x over previous
"""Trainium2 Bass kernel for nn_LiquidGenerator.

score = sum over (i, image j) pairs of (CUTOFF - dist)^2 where dist < CUTOFF,
with dist over the [N, 27N] supercell distance matrix.

Strategy
--------
Host (O(N) prep): generate P (rotation+translation of molecule-local coords,
float64), exploit the shift symmetry d(i,(k,j)) == d(j,(-k,i)):
    score = sum_full(central) + 2 * sum_full(13 half-shifts)
so only 14 of the 27 images are computed. Distances come from a 5-feature
inner product:  d^2 + BIAS = [Px,Py,Pz,|P|^2,1] . [-2Sx,-2Sy,-2Sz, 1, |S|^2+BIAS]
(coordinates centered at the cell midpoint for fp32 accuracy; BIAS keeps the
PE-accumulated value strictly positive so ACT Sqrt never sees a negative).

Device (8 NeuronCores, j-sharded): each core takes 128 columns of each of the
14 shift blocks (1792 cols) for all 1024 i. Per 128-row i-chunk:
  - TensorE: 4 fp32 matmuls (K=5) -> PSUM [128,1792] holding d^2+BIAS
  - ScalarE: Sqrt activation PSUM->SBUF
  - VectorE: one fused custom-DVE op per weight group:
        sq(relu(CUTOFF - s)) * ne(Idx, diag) * weight, accum=ADD
    (free reduction to [128,1]; diag masks the self-pairs, which are added
    back exactly on the host).
Host: fp64 sum of the 8x[128,16] partials + exact self-pair term.
"""

import numpy as np

CUTOFF = 3.0
EPS = 1e-16
BIAS = 2e-4

NCORES = 8
N = 1024           # 128 molecules x 8 atoms
NSH = 14           # central + 13 half-shifts
JC = N // NCORES   # 128 j-columns per shift block per core
W = NSH * JC       # 1792 columns per core
NCHUNK = 8         # i-chunks of 128

_OP_NAME = "LIQUID_SCORE_ANT"
_cache: dict = {}


# ----------------------------------------------------------------- host math
def _rotation_matrices(rot):
    a, b, g = rot[:, 0], rot[:, 1], rot[:, 2]
    ca, sa = np.cos(a), np.sin(a)
    cb, sb = np.cos(b), np.sin(b)
    cg, sg = np.cos(g), np.sin(g)
    m = rot.shape[0]
    rx = np.zeros((m, 3, 3)); ry = np.zeros((m, 3, 3)); rz = np.zeros((m, 3, 3))
    rx[:, 0, 0] = 1;  rx[:, 1, 1] = ca; rx[:, 1, 2] = -sa; rx[:, 2, 1] = sa; rx[:, 2, 2] = ca
    ry[:, 0, 0] = cb; ry[:, 0, 2] = -sb; ry[:, 1, 1] = 1;  ry[:, 2, 0] = sb; ry[:, 2, 2] = cb
    rz[:, 0, 0] = cg; rz[:, 0, 1] = -sg; rz[:, 1, 0] = sg; rz[:, 1, 1] = cg; rz[:, 2, 2] = 1
    return np.einsum("mij,mjk,mkl->mil", rx, ry, rz)


def _generate(positions, translation, rotation, cell):
    R = _rotation_matrices(rotation.astype(np.float64))
    trans = np.remainder(translation.astype(np.float64), 1.0) @ cell.astype(np.float64)
    gen = np.einsum("mai,mij->maj", positions.astype(np.float64), R) + trans[:, None, :]
    return gen.reshape(-1, 3)


# ------------------------------------------------------------- bass program
def _build_program(reps: int = 1):
    key = ("nc", reps)
    if key in _cache:
        return _cache[key]
    from contextlib import ExitStack
    import concourse.bass as bass  # noqa: F401
    import concourse.tile as tile
    from concourse import bacc, mybir

    f32 = mybir.dt.float32
    # Bacc (not raw Bass): its compile() runs the wait-legalization passes
    # (move_matmul_waits_to_ldweights / generate_event_semaphores) that split
    # multi-semaphore waits, plus ACT table-load insertion.
    nc = bacc.Bacc("TRN2", target_bir_lowering=False, debug=False, num_devices=NCORES)
    # lhsT [5, N] and rhs [5, W] bundled: a single DMA -> a single sync wait on
    # the first matmul (a matmul can encode only ONE sync wait; two DMA sems
    # overflow it at walrus codegen). Tail of 64 zero columns doubles as the
    # bf16-zero operand pool for the toucher matmuls.
    feat_d = nc.dram_tensor("feat", [5, N + W + 64], f32, kind="ExternalInput")
    acc_d = nc.dram_tensor("acc", [128, 2 * NCHUNK], f32, kind="ExternalOutput")
    sdiag_d = nc.dram_tensor("sdiag", [128, NCHUNK * JC], f32, kind="ExternalOutput")

    with tile.TileContext(nc) as tc, ExitStack() as ctx:
        const = ctx.enter_context(tc.tile_pool(name="const", bufs=1))
        psum = ctx.enter_context(tc.tile_pool(name="psum", bufs=2, space="PSUM"))
        # one st buffer per chunk: no SBUF reuse -> no cross-engine WAR deps
        # (this walrus build allows at most ONE sync wait per instruction)
        spool = ctx.enter_context(tc.tile_pool(name="s", bufs=NCHUNK))
        scrap = ctx.enter_context(tc.tile_pool(name="scrap", bufs=2))

        ft = const.tile([5, N + W + 64], f32)
        nc.sync.dma_start(ft[:], feat_d[:])
        lt = ft[:, 0:N]
        rt = ft[:, N:N + W]
        at = const.tile([128, 2 * NCHUNK], f32)
        sall = const.tile([128, NCHUNK * JC], f32)

        # bf16-zero views of the zero-padded feat tail for "toucher" matmuls
        bw = ft[0:1, N + W:N + W + 64].bitcast(mybir.dt.bfloat16)  # [1,128] bf16 zeros
        bx = bw[:, 0:1]

        for ic in range(NCHUNK * reps):
            ic = ic % NCHUNK
            ps = psum.tile([128, W], f32)
            for o in range(0, W, 512):
                w = min(512, W - o)
                nc.tensor.matmul(
                    ps[:, o:o + w],
                    lt[:, ic * 128:(ic + 1) * 128],
                    rt[:, o:o + w],
                    start=True, stop=True,
                )
            st = spool.tile([128, W], f32)
            nc.scalar.activation(st[:], ps[:], mybir.ActivationFunctionType.Sqrt)
            # Toucher: after ACT has read the PSUM tile, a 1-column bf16 matmul
            # re-takes ownership on the PE with a single ACT wait. Walrus allows
            # at most ONE sync wait per matmul; without this, the next chunk's
            # fp32 matmul would need two (ACT done + PE completion).
            nc.tensor.matmul(ps[:, 0:1], bw, bx, start=True, stop=True)
            # stash central s block for exact host-side self-pair correction
            # (GPSIMD is otherwise idle; >8 DMAs would reuse HW queues and need
            # a second sync wait, which walrus rejects)
            nc.gpsimd.tensor_copy(sall[:, ic * JC:(ic + 1) * JC], st[:, 0:JC])
            # v = min(s, 3) - 3  ->  v^2 == relu(3-s)^2
            vt = scrap.tile([128, W], f32)
            nc.vector.tensor_scalar(
                vt[:], st[:], CUTOFF, CUTOFF,
                mybir.AluOpType.min, mybir.AluOpType.subtract,
            )
            sq = scrap.tile([128, W], f32, tag="sqout")
            # central block (weight 1 on host)
            nc.vector.scalar_tensor_tensor(
                sq[:, 0:JC], vt[:, 0:JC], 1.0, vt[:, 0:JC],
                mybir.AluOpType.mult, mybir.AluOpType.mult,
                accum_out=at[:, 2 * ic:2 * ic + 1],
            )
            # 13 half-shift blocks (weight 2 on host, via shift symmetry)
            nc.vector.scalar_tensor_tensor(
                sq[:, JC:W], vt[:, JC:W], 1.0, vt[:, JC:W],
                mybir.AluOpType.mult, mybir.AluOpType.mult,
                accum_out=at[:, 2 * ic + 1:2 * ic + 2],
            )
        nc.sync.dma_start(acc_d[:], at[:])
        nc.sync.dma_start(sdiag_d[:], sall[:])

    # Bacc.finalize runs compile(): wait legalization (one sync wait per
    # instruction on this walrus), ACT table loads, register allocation.
    nc.finalize()
    _cache[key] = nc
    return nc


# --------------------------------------------------------------- input prep
def _prepare_inputs(positions, translation, rotation, cell):
    cell64 = cell.astype(np.float64)
    P = _generate(positions, translation, rotation, cell64)      # [N,3] float64
    n = P.shape[0]
    assert n == N, f"kernel hardcodes N={N}, got {n}"

    shifts = np.array([-1.0, 0.0, 1.0])
    offs = np.stack(np.meshgrid(shifts, shifts, shifts, indexing="ij")).reshape(3, -1).T
    vecs = offs @ cell64                                          # [27,3]
    assert np.all(offs[13] == 0.0)
    used = [13] + [k for k in range(13)]                          # central first

    c = 0.5 * cell64.sum(axis=0)
    Pc = (P - c).astype(np.float32)
    lhsT = np.stack([
        Pc[:, 0], Pc[:, 1], Pc[:, 2],
        (Pc.astype(np.float64) ** 2).sum(1).astype(np.float32),
        np.ones(n, np.float32),
    ]).astype(np.float32)                                         # [5, N]

    rhs_blocks = []
    for k in used:
        S = (P + vecs[k] - c).astype(np.float32)
        rhs_blocks.append(np.stack([
            -2.0 * S[:, 0], -2.0 * S[:, 1], -2.0 * S[:, 2],
            np.ones(n, np.float32),
            (S.astype(np.float64) ** 2).sum(1).astype(np.float32) + np.float32(BIAS),
        ]).astype(np.float32))                                    # [5, N] each

    in_maps = []
    for core in range(NCORES):
        sl = slice(core * JC, (core + 1) * JC)
        rhs = np.concatenate([b[:, sl] for b in rhs_blocks], axis=1)  # [5, W]
        in_maps.append({
            "feat": np.ascontiguousarray(np.concatenate(
                [lhsT, rhs, np.zeros((5, 64), np.float32)], axis=1)),
        })
    return in_maps


LAST_RESULTS = None


def _get_runner(reps: int = 1):
    """Jit the bass program once; reuse the compiled executable per call.

    (bass2jax.run_bass_via_pjrt rebuilds the jit closure every call, paying
    retrace + executable reload each time.)
    """
    key = ("runner", reps)
    if key in _cache:
        return _cache[key]
    import jax
    import numpy as jnp_np  # noqa
    from jax.sharding import Mesh, PartitionSpec
    from jax.experimental.shard_map import shard_map
    from concourse import bass2jax, mybir

    nc = _build_program(reps=reps)
    bass2jax.install_neuronx_cc_hook()

    partition_name = (
        nc.partition_id_tensor.name if nc.partition_id_tensor else None
    )
    in_names, out_names, out_avals, zero_outs = [], [], [], []
    for alloc in nc.m.functions[0].allocations:
        if not isinstance(alloc, mybir.MemoryLocationSet):
            continue
        name = alloc.memorylocations[0].name
        if alloc.kind == "ExternalInput":
            if name != partition_name:
                in_names.append(name)
        elif alloc.kind == "ExternalOutput":
            out_names.append(name)
            shape = tuple(alloc.tensor_shape)
            dtype = mybir.dt.np(alloc.dtype)
            out_avals.append(jax.core.ShapedArray(shape, dtype))
            zero_outs.append(np.zeros(shape, dtype))
    n_params = len(in_names)
    all_in_names = in_names + out_names
    if partition_name is not None:
        all_in_names = all_in_names + [partition_name]

    def _body(*args):
        operands = list(args)
        if partition_name is not None:
            operands.append(bass2jax.partition_id_tensor())
        outs = bass2jax._bass_exec_p.bind(
            *operands,
            out_avals=tuple(out_avals),
            in_names=tuple(all_in_names),
            out_names=tuple(out_names),
            lowering_input_output_aliases=(),
            sim_require_finite=True,
            sim_require_nnan=True,
            nc=nc,
        )
        return tuple(outs)

    devices = jax.devices()[:NCORES]
    mesh = Mesh(np.asarray(devices), ("core",))
    n_outs = len(out_names)
    sharded = jax.jit(
        shard_map(
            _body, mesh=mesh,
            in_specs=(PartitionSpec("core"),) * (n_params + n_outs),
            out_specs=(PartitionSpec("core"),) * n_outs,
            check_rep=False,
        ),
        keep_unused=True,
    )
    concat_zeros = [
        np.zeros((NCORES * z.shape[0], *z.shape[1:]), z.dtype) for z in zero_outs
    ]

    def run(in_maps):
        concat_in = [
            np.concatenate([in_maps[c][name] for c in range(NCORES)], axis=0)
            for name in in_names
        ]
        out_arrs = sharded(*concat_in, *concat_zeros)
        return [
            {
                name: np.asarray(out_arrs[i]).reshape(NCORES, *out_avals[i].shape)[c]
                for i, name in enumerate(out_names)
            }
            for c in range(NCORES)
        ]

    _cache[key] = run
    return run


class _Res:
    def __init__(self, results):
        self.results = results


def kernel(positions, translation, rotation, cell, _reps=1):
    global LAST_RESULTS
    run = _get_runner(reps=_reps)
    in_maps = _prepare_inputs(
        np.asarray(positions), np.asarray(translation),
        np.asarray(rotation), np.asarray(cell),
    )
    res = _Res(run(in_maps))
    LAST_RESULTS = res
    total = 0.0
    for core, r in enumerate(res.results):
        acc = r["acc"].astype(np.float64)
        total += acc[:, 0::2].sum() + 2.0 * acc[:, 1::2].sum()
        # exact removal of the device-computed self-pair terms: the diagonal of
        # this core's central block, recomputed from the exact s values
        s_pp = r["sdiag"][:, core * JC:(core + 1) * JC].diagonal()
        v = (np.minimum(s_pp, np.float32(CUTOFF)) - np.float32(CUTOFF)).astype(np.float32)
        total -= (v.astype(np.float64) ** 2).sum()
    total += N * (CUTOFF - np.sqrt(np.float32(EPS))) ** 2  # exact self pairs
    return np.float32(total)
